# revision 32
# baseline (speedup 1.0000x reference)
# Causal GQA self-attention on 8 TRN2 NeuronCores (Bass/Tile, SPMD).
#
# Sharding: core c -> (batch b = c//4, head-group g = c%4). Each core computes
# q-heads 4g..4g+3 with kv-head g for its batch, then AllToAll re-shards
# attention outputs from head-split to token-split; core c runs the output
# projection for token rows [256c, 256c+256) of BOTH batches with full Wo.
#
# v3: all matmul operands bf16 (host-cast) - halves HBM traffic and avoids the
# fp32r board-power throttle (GPIO 13/16 cap seen in NTFF). Exps are paired to
# [128,1024] tiles to amortize the ACT engine's fixed per-instruction cost
# (cayman errata makes ACT ~2x slower than spec). Softmax denominators ride
# the PE's slack in the ACT-bound attention phase (ones-matmul accumulation).
# DMAs are batched into single multi-dim-AP transfers (one per x chunk, one
# per weight set, one for all of Wo, one per attention-out store) so engine
# queues don't burn ~650ns per trigger. The AllToAll is split in three
# (heads 0-1 / 2 / 3) and o_proj runs in three passes with SBUF partials so
# the PE rolls through collectives without idling (HAM re-throttles to
# 1.2GHz after ~3.4us idle). Warm-up dummy matmuls cover the initial DMA
# wait; v transposes use the 16-bit DMA XBAR path instead of the PE.
import numpy as np

B, T, C = 2, 2048, 2048
H, KV, HD = 16, 4, 128
NCORES = 8
INV_SQRT_HD = 1.0 / float(np.sqrt(HD))
EXP_BIAS = -np.log(16.0)  # keeps denominator sums small; cancels in normalize

_cache = {}


def _build(t_len, c_len):
    import concourse.bass as bass  # noqa: F401
    import concourse.mybir as mybir
    import concourse.tile as tile
    from concourse import bacc
    from concourse.masks import make_identity

    F32 = mybir.dt.float32
    BF = mybir.dt.bfloat16
    F16 = mybir.dt.float16
    AF = mybir.ActivationFunctionType
    MUL = mybir.AluOpType.mult
    ADD = mybir.AluOpType.add

    NT = t_len // 128          # token tiles
    NC_ = c_len // 128         # channel tiles
    NCH = t_len // 512         # 512-wide token chunks
    HL = 4                     # local q heads
    TS = t_len // 8            # per-core token slice for o_proj

    nc = bacc.Bacc("TRN2", target_bir_lowering=False, debug=False,
                   num_devices=NCORES)

    xT_ap = nc.dram_tensor("xT", [c_len, t_len], BF, kind="ExternalInput").ap()
    wq_ap = nc.dram_tensor("wq", [c_len, 512], BF, kind="ExternalInput").ap()
    wk_ap = nc.dram_tensor("wk", [c_len, 128], BF, kind="ExternalInput").ap()
    wv_ap = nc.dram_tensor("wv", [c_len, 128], BF, kind="ExternalInput").ap()
    wo_ap = nc.dram_tensor("wo", [2048, 2048], BF, kind="ExternalInput").ap()
    cos_ap = nc.dram_tensor("cosT", [128, t_len], BF, kind="ExternalInput").ap()
    sin_ap = nc.dram_tensor("sinTs", [128, t_len], BF, kind="ExternalInput").ap()
    msk_ap = nc.dram_tensor("masks", [2 * 128, 1024], F16, kind="ExternalInput").ap()
    o_ap = nc.dram_tensor("o", [2 * 2048, TS], F32, kind="ExternalOutput").ap()
    # head-split -> token-split shuffles: heads 0-1 together, heads 2,3 alone
    a2i_ab = nc.dram_tensor("a2i_ab", [NCORES * 256, TS], BF).ap()
    a2o_ab = nc.dram_tensor("a2o_ab", [NCORES * 256, TS], BF).ap()
    a2i_b = nc.dram_tensor("a2i_b", [NCORES * 128, TS], BF).ap()
    a2o_b = nc.dram_tensor("a2o_b", [NCORES * 128, TS], BF).ap()
    a2i_c = nc.dram_tensor("a2i_c", [NCORES * 128, TS], BF).ap()
    a2o_c = nc.dram_tensor("a2o_c", [NCORES * 128, TS], BF).ap()

    def a2a(in_ap, out_ap):
        nc.gpsimd.collective_compute(
            "AllToAll", mybir.AluOpType.bypass,
            replica_groups=[list(range(NCORES))],
            ins=[in_ap[:, :]], outs=[out_ap[:, :]],
        )

    with tile.TileContext(nc) as tc:
        with tc.tile_pool(name="const", bufs=1) as constp:
            ones_k = constp.tile([128, 1], F16)
            nc.vector.memset(ones_k[:, :], 1.0)
            ones_m = constp.tile([1, 128], BF)
            nc.vector.memset(ones_m[:, :], 1.0)
            dum_w = constp.tile([128, 128], BF)
            nc.vector.memset(dum_w[:, :], 0.0)
            dum_x = constp.tile([128, 512], BF)
            nc.vector.memset(dum_x[:, :], 0.0)
            ebias = constp.tile([128, 1], F32)
            nc.vector.memset(ebias[:, :], EXP_BIAS)
            idt = constp.tile([128, 128], F16)
            make_identity(nc, idt[:, :])

            with tc.tile_pool(name="act", bufs=1) as pp:
                # persistent activations (freed before the o_proj phase)
                qT = [pp.tile([128, t_len], BF, tag=f"qT{j}", name=f"qT{j}")
                      for j in range(HL)]
                kT = pp.tile([128, t_len], BF)
                v_t = [pp.tile([128, 128], F16, tag=f"v{tt}", name=f"v{tt}")
                       for tt in range(NT)]
                masks2 = [pp.tile([128, 1024], F16, tag=f"msk{i}", name=f"msk{i}")
                          for i in range(2)]
                rhs_ab = pp.tile([128, 8, 2, TS], BF, tag="rab", name="rab")
                rhs_b = pp.tile([128, 8, TS], BF, tag="rb", name="rb")
                cosT = pp.tile([128, t_len], BF)
                sinTs = pp.tile([128, t_len], BF)
                # Wo resident for o_proj: [128, 16, 2048] (64KB/partition)
                wo_sb = pp.tile([128, 16, 2048], BF, tag="wo", name="wo_sb")

                # ------------ phase 1+2: projections + RoPE -----------------
                with (
                    tc.tile_pool(name="ph2", bufs=1) as ph2,
                    tc.tile_pool(name="ph2ps", bufs=2, space="PSUM") as ph2ps,
                ):
                    # warm the PE (HAM un-throttles after ~3.4us of activity)
                    ps_w = ph2ps.tile([128, 512], F32, tag="acc", name="ps_w")
                    for _ in range(8):
                        nc.tensor.matmul(out=ps_w[:, :], lhsT=dum_w[:, :],
                                         rhs=dum_x[:, :], start=True, stop=True)
                    # weights + tables on the scalar DMA queue, x on sync.
                    # x chunk / wq arrive in 4-ct quarters so compute can
                    # start before the whole chunk lands.
                    wq_sb = ph2.tile([128, NC_, 512], BF, tag="wq", name="wq")
                    wk_sb = ph2.tile([128, NC_, 128], BF, tag="wk", name="wk")
                    wv_sb = ph2.tile([128, NC_, 128], BF, tag="wv", name="wv")
                    xs0 = ph2.tile([128, NC_, 512], BF, tag="xs", name="xs0",
                                   bufs=2)
                    for q4 in range(0, NC_, 4):
                        n4 = min(4, NC_ - q4)
                        nc.scalar.dma_start(
                            out=wq_sb[:, q4:q4+n4, :],
                            in_=wq_ap[q4*128:(q4+n4)*128, :].rearrange(
                                "(ct p) w -> p ct w", p=128))
                        eng = nc.sync if (q4 // 4) % 2 == 0 else nc.scalar
                        eng.dma_start(
                            out=xs0[:, q4:q4+n4, :],
                            in_=xT_ap[q4*128:(q4+n4)*128, 0:512].rearrange(
                                "(ct p) t -> p ct t", p=128))
                    nc.scalar.dma_start(
                        out=wk_sb[:, :, :],
                        in_=wk_ap[:, :].rearrange("(ct p) w -> p ct w", p=128))
                    nc.scalar.dma_start(
                        out=wv_sb[:, :, :],
                        in_=wv_ap[:, :].rearrange("(ct p) w -> p ct w", p=128))
                    nc.scalar.dma_start(out=cosT[:, :], in_=cos_ap[:, :])
                    nc.scalar.dma_start(out=sinTs[:, :], in_=sin_ap[:, :])
                    for i in range(2):
                        nc.scalar.dma_start(out=masks2[i][:, :],
                                            in_=msk_ap[i*128:(i+1)*128, :])

                    for ch in range(NCH):
                        sl = slice(ch * 512, (ch + 1) * 512)
                        if ch == 0:
                            xs = xs0
                        else:
                            xs = ph2.tile([128, NC_, 512], BF, tag="xs",
                                          name=f"xs{ch}", bufs=2)
                            for q4 in range(0, NC_, 4):
                                n4 = min(4, NC_ - q4)
                                eng = (nc.sync if (q4 // 4) % 2 == 0
                                       else nc.scalar)
                                eng.dma_start(
                                    out=xs[:, q4:q4+n4, :],
                                    in_=xT_ap[q4*128:(q4+n4)*128, sl].rearrange(
                                        "(ct p) t -> p ct t", p=128))
                        # q heads + k: project, then RoPE
                        for u in range(HL + 1):
                            ps_a = ph2ps.tile([128, 512], F32, tag="acc", name="ps_a")
                            for ct in range(NC_):
                                w = (wq_sb[:, ct, u*128:(u+1)*128] if u < HL
                                     else wk_sb[:, ct, :])
                                nc.tensor.matmul(out=ps_a[:, :], lhsT=w,
                                                 rhs=xs[:, ct, :],
                                                 start=(ct == 0), stop=(ct == NC_ - 1))
                            raw = ph2.tile([128, 512], BF, tag="raw", bufs=2, name="raw")
                            nc.scalar.activation(raw[:, :], ps_a[:, :], AF.Copy)
                            sw = ph2.tile([128, 512], BF, tag="sw", bufs=2, name="sw")
                            nc.sync.dma_start(out=sw[0:64, :], in_=raw[64:128, :])
                            nc.sync.dma_start(out=sw[64:128, :], in_=raw[0:64, :])
                            t1 = ph2.tile([128, 512], BF, tag="t1", bufs=2, name="t1")
                            nc.vector.tensor_tensor(t1[:, :], sw[:, :], sinTs[:, sl], MUL)
                            t2 = ph2.tile([128, 512], BF, tag="t2", bufs=2, name="t2")
                            nc.vector.tensor_tensor(t2[:, :], raw[:, :], cosT[:, sl], MUL)
                            dst = qT[u][:, sl] if u < HL else kT[:, sl]
                            nc.vector.tensor_tensor(dst, t1[:, :], t2[:, :], ADD)
                        # v: project then transpose to token-major (DMA XBAR)
                        ps_a = ph2ps.tile([128, 512], F32, tag="acc", name="ps_av")
                        for ct in range(NC_):
                            nc.tensor.matmul(out=ps_a[:, :], lhsT=wv_sb[:, ct, :],
                                             rhs=xs[:, ct, :],
                                             start=(ct == 0), stop=(ct == NC_ - 1))
                        vraw = ph2.tile([128, 512], F16, tag="vraw", bufs=2, name="vraw")
                        nc.scalar.activation(vraw[:, :], ps_a[:, :], AF.Copy)
                        for tt4 in range(4):
                            ps_tr = ph2ps.tile([128, 128], F16, tag="tr",
                                               bufs=2, name="ps_tr")
                            nc.tensor.transpose(ps_tr[:, :],
                                                vraw[:, tt4*128:(tt4+1)*128],
                                                idt[:, :])
                            nc.vector.tensor_copy(v_t[ch*4+tt4][:, :],
                                                  ps_tr[:, :])
                        if ch < 2:
                            ps_d = ph2ps.tile([128, 512], F32, tag="acc",
                                              name=f"ps_d{ch}")
                            for _ in range(6):
                                nc.tensor.matmul(out=ps_d[:, :],
                                                 lhsT=dum_w[:, :],
                                                 rhs=dum_x[:, :],
                                                 start=True, stop=True)

                # ---------------- phase 3: attention (4 heads x NCH chunks)
                with (
                    tc.tile_pool(name="ph3", bufs=1) as ph3,
                    tc.tile_pool(name="ph3ps", bufs=1, space="PSUM") as ph3ps,
                ):
                    for h in range(HL):
                        for ch in range(NCH):
                            sl = slice(ch * 512, (ch + 1) * 512)
                            njt = 4 * ch + 4
                            npair = njt // 2
                            ps_av = ph3ps.tile([128, 512], F32, tag="av", bufs=2,
                                               name="ps_av3")
                            # denominator running sum off the PE (DVE)
                            asum = ph3.tile([128, 1024], F16, tag="asum",
                                            bufs=2, name="asum")
                            att = []
                            # software pipeline: scores/exp run a pair ahead
                            # of the AV accumulation
                            for pr in range(npair + 1):
                                if pr < npair:
                                    ps_s = ph3ps.tile([128, 1024], F32, tag="s",
                                                      bufs=2, name="ps_s")
                                    for half in range(2):
                                        jt = 2 * pr + half
                                        nc.tensor.matmul(
                                            out=ps_s[:, half*512:(half+1)*512],
                                            lhsT=kT[:, jt*128:(jt+1)*128],
                                            rhs=qT[h][:, sl],
                                            start=True, stop=True)
                                    a2 = ph3.tile([128, 1024], F16,
                                                  tag=f"a{pr % 2}", bufs=2,
                                                  name=f"a{pr}")
                                    nc.scalar.activation(a2[:, :], ps_s[:, :],
                                                         AF.Exp,
                                                         scale=INV_SQRT_HD,
                                                         bias=ebias[:, :])
                                    if pr >= 2 * ch:
                                        am = ph3.tile([128, 1024], F16,
                                                      tag=f"am{pr % 2}", bufs=2,
                                                      name=f"am{pr}")
                                        nc.vector.tensor_tensor(
                                            am[:, :], a2[:, :],
                                            masks2[pr - 2 * ch][:, :], MUL)
                                        a2 = am
                                    att.append(a2)
                                    if pr == 0:
                                        nc.vector.tensor_copy(asum[:, :],
                                                              a2[:, :])
                                    else:
                                        nc.vector.tensor_tensor(
                                            asum[:, :], asum[:, :],
                                            a2[:, :], ADD)
                                if pr > 0:
                                    prev = att[pr - 1]
                                    for half in range(2):
                                        jt = 2 * (pr - 1) + half
                                        pa = prev[:, half*512:(half+1)*512]
                                        nc.tensor.matmul(
                                            out=ps_av[:, :], lhsT=v_t[jt][:, :],
                                            rhs=pa, start=(jt == 0),
                                            stop=(jt == njt - 1))
                            # collapse asum on the PE: two small
                            # accumulating matmuls -> [1,512]
                            ps_dn = ph3ps.tile([1, 512], F32, tag="den", bufs=1,
                                               name="ps_dn")
                            for half in range(2):
                                nc.tensor.matmul(
                                    out=ps_dn[:, :], lhsT=ones_k[:, :],
                                    rhs=asum[:, half*512:(half+1)*512],
                                    start=(half == 0), stop=(half == 1))
                            dnb = ph3.tile([1, 512], BF, tag="dnb", bufs=2,
                                           name="dnb")
                            nc.vector.tensor_copy(dnb[:, :], ps_dn[:, :])
                            if h == 0 and ch == 0:
                                # WAW gate: a 1-element write into wo_sb that
                                # reads the tail of kT forces the 8MB Wo
                                # stream to wait until the projection phase's
                                # x loads are done (else it steals their HBM
                                # bandwidth at kernel start)
                                nc.gpsimd.tensor_copy(
                                    wo_sb[0:1, 0:1, 0:1],
                                    kT[0:1, t_len-1:t_len])
                                nc.gpsimd.dma_start(
                                    out=wo_sb[:, :, :],
                                    in_=wo_ap[:, :].rearrange(
                                        "(jt p) w -> p jt w", p=128))
                            ps_bc = ph3ps.tile([128, 512], F32, tag="bc", bufs=1,
                                               name="ps_bc")
                            nc.tensor.matmul(out=ps_bc[:, :], lhsT=ones_m[:, :],
                                             rhs=dnb[:, :], start=True, stop=True)
                            rec = ph3.tile([128, 512], F32, tag="rec", bufs=2,
                                           name="rec")
                            nc.vector.reciprocal_approx_fast(rec[:, :], ps_bc[:, :])
                            ao = ph3.tile([128, 512], BF, tag="ao", bufs=2,
                                          name="ao")
                            nc.vector.tensor_tensor(ao[:, :], ps_av[:, :],
                                                    rec[:, :], MUL)
                            # one store per (h,ch): all TS-token slices at once
                            nhf = 512 // TS
                            if h < 2:
                                blk = a2i_ab[ch*nhf*256:(ch+1)*nhf*256, :]
                                dst = blk.rearrange(
                                    "(hf hh r) t -> r hh hf t", hh=2,
                                    r=128)[:, h, :, :]
                            elif h == 2:
                                blk = a2i_b[ch*nhf*128:(ch+1)*nhf*128, :]
                                dst = blk.rearrange("(hf r) t -> r hf t",
                                                    r=128)
                            else:
                                blk = a2i_c[ch*nhf*128:(ch+1)*nhf*128, :]
                                dst = blk.rearrange("(hf r) t -> r hf t",
                                                    r=128)
                            nc.sync.dma_start(
                                out=dst,
                                in_=ao[:, :].rearrange("r (hf t) -> r hf t",
                                                       hf=nhf))
                        if h == 1:
                            a2a(a2i_ab, a2o_ab)
                            nc.gpsimd.dma_start(
                                out=rhs_ab[:, :, :, :],
                                in_=a2o_ab[:, :].rearrange(
                                    "(s hh p) t -> p s hh t", s=8, hh=2))
                        elif h == 2:
                            a2a(a2i_b, a2o_b)
                            nc.gpsimd.dma_start(
                                out=rhs_b[:, :, :],
                                in_=a2o_b[:, :].rearrange(
                                    "(s p) t -> p s t", s=8))
                        elif h == 3:
                            a2a(a2i_c, a2o_c)

                # -------- phase 4: o_proj, three passes with SBUF partials
                with (
                    tc.tile_pool(name="ph4", bufs=1) as ph4,
                    tc.tile_pool(name="ph4ps", bufs=2, space="PSUM") as ph4ps,
                ):
                    rhs_c = ph4.tile([128, 8, TS], BF, tag="rc", name="rc")
                    nc.gpsimd.dma_start(
                        out=rhs_c[:, :, :],
                        in_=a2o_c[:, :].rearrange("(s p) t -> p s t", s=8))

                    def rhs_for(jt, bb):
                        src = 4 * bb + jt // 4
                        hh = jt % 4
                        if hh < 2:
                            return rhs_ab[:, src, hh, :]
                        return (rhs_b if hh == 2 else rhs_c)[:, src, :]

                    JTAB = [jt for jt in range(16) if jt % 4 < 2]
                    JTC2 = [jt for jt in range(16) if jt % 4 == 2]
                    JTC3 = [jt for jt in range(16) if jt % 4 == 3]
                    part = [ph4.tile([128, 2 * TS], F32, tag=f"pt{cc}",
                                     name=f"pt{cc}") for cc in range(16)]
                    for cc in range(16):
                        ps_o = ph4ps.tile([128, 2 * TS], F32, tag="o", name="ps_o")
                        for idx, jt in enumerate(JTAB):
                            for bb in range(2):
                                nc.tensor.matmul(
                                    out=ps_o[:, bb*TS:(bb+1)*TS],
                                    lhsT=wo_sb[:, jt, cc*128:(cc+1)*128],
                                    rhs=rhs_for(jt, bb),
                                    start=(idx == 0), stop=(idx == 7))
                        nc.scalar.activation(part[cc][:, :], ps_o[:, :], AF.Copy)
                    for cc in range(16):
                        ps_o = ph4ps.tile([128, 2 * TS], F32, tag="o", name="ps_o2")
                        for idx, jt in enumerate(JTC2):
                            for bb in range(2):
                                nc.tensor.matmul(
                                    out=ps_o[:, bb*TS:(bb+1)*TS],
                                    lhsT=wo_sb[:, jt, cc*128:(cc+1)*128],
                                    rhs=rhs_for(jt, bb),
                                    start=(idx == 0), stop=(idx == 3))
                        nc.vector.tensor_tensor(part[cc][:, :], ps_o[:, :],
                                                part[cc][:, :], ADD)
                    for cc in range(16):
                        ps_o = ph4ps.tile([128, 2 * TS], F32, tag="o", name="ps_o3")
                        for idx, jt in enumerate(JTC3):
                            for bb in range(2):
                                nc.tensor.matmul(
                                    out=ps_o[:, bb*TS:(bb+1)*TS],
                                    lhsT=wo_sb[:, jt, cc*128:(cc+1)*128],
                                    rhs=rhs_for(jt, bb),
                                    start=(idx == 0), stop=(idx == 3))
                        osb = ph4.tile([128, 2 * TS], F32, tag="osb", bufs=2,
                                       name="osb")
                        nc.vector.tensor_tensor(osb[:, :], ps_o[:, :],
                                                part[cc][:, :], ADD)
                        nc.sync.dma_start(
                            out=o_ap[:, :].rearrange(
                                "(bb q r) t -> r q bb t", bb=2,
                                r=128)[:, cc, :, :],
                            in_=osb[:, :].rearrange("r (bb t) -> r bb t",
                                                    bb=2))

    nc.compile()
    return nc


def _prep_inputs(x, cos, sin, Wq, Wk, Wv, Wo):
    import ml_dtypes
    BF = ml_dtypes.bfloat16

    x = np.asarray(x, dtype=np.float32)
    cos = np.asarray(cos, dtype=np.float32)
    sin = np.asarray(sin, dtype=np.float32)

    t_len = x.shape[1]
    cosT = np.ascontiguousarray(cos.T).astype(BF)            # [128, T]
    sinT = np.asarray(sin.T, dtype=np.float32).copy()
    sinT[0:64, :] *= -1.0                                    # signed swap-half
    sinTs = np.ascontiguousarray(sinT).astype(BF)

    tk = np.arange(128)[:, None]
    tq = np.arange(512)[None, :]
    m4 = np.zeros((4, 128, 512), dtype=np.float32)
    for jd in range(4):
        m4[jd] = (128 * jd + tk <= tq).astype(np.float32)
    # paired: [2, 128, 1024] = concat of tiles (0,1) and (2,3) along cols
    masks = np.concatenate(
        [np.concatenate([m4[2*k], m4[2*k+1]], axis=1)[None] for k in range(2)],
        axis=0).reshape(2 * 128, 1024).astype(np.float16)

    wo_bf = np.ascontiguousarray(np.asarray(Wo, dtype=np.float32)).astype(BF)
    wq_f = np.asarray(Wq, dtype=np.float32)
    wk_f = np.asarray(Wk, dtype=np.float32)
    wv_f = np.asarray(Wv, dtype=np.float32)

    in_maps = []
    for c in range(NCORES):
        b, g = c // 4, c % 4
        xb = x[b] if x.ndim == 3 else x
        in_maps.append({
            "xT": np.ascontiguousarray(xb.T).astype(BF),
            "wq": np.ascontiguousarray(wq_f[:, 512*g:512*(g+1)]).astype(BF),
            "wk": np.ascontiguousarray(wk_f[:, 128*g:128*(g+1)]).astype(BF),
            "wv": np.ascontiguousarray(wv_f[:, 128*g:128*(g+1)]).astype(BF),
            "wo": wo_bf,
            "cosT": cosT,
            "sinTs": sinTs,
            "masks": masks,
        })
    return in_maps, t_len


def kernel(x, cos, sin, Wq, Wk, Wv, Wo):
    from concourse.bass_utils import run_bass_kernel_spmd

    in_maps, t_len = _prep_inputs(x, cos, sin, Wq, Wk, Wv, Wo)
    c_len = in_maps[0]["xT"].shape[0]
    key = (t_len, c_len)
    if key not in _cache:
        _cache[key] = _build(t_len, c_len)
    nc = _cache[key]

    res = run_bass_kernel_spmd(nc, in_maps, core_ids=list(range(NCORES)))
    ts = t_len // 8
    out = np.empty((2, t_len, 2048), dtype=np.float32)
    for c in range(NCORES):
        o = res.results[c]["o"]
        out[0, ts*c:ts*(c+1), :] = o[0:2048].T
        out[1, ts*c:ts*(c+1), :] = o[2048:4096].T
    return out
